# revision 28
# baseline (speedup 1.0000x reference)
"""CCAMDec (channel-attention decoder) Trainium2 Bass kernel.

Data-parallel over batch N=8 across 8 NeuronCores (one batch per core).
Per core (C=512, K=64, HW=4096):
  energy[c,k]   = sum_s x[c,s] * y[k,s]         (fp16 matmul, fp32 accum)
  att[c,k]      = softmax_k(max_k(E) - E)       (== exp(min_k(E)-E)/sum)
  out[c,s]      = x[c,s] + scale * sum_k att[c,k] y[k,s]

The kernel is HBM-bandwidth-bound (17.8MB of f32 I/O per core at
~358GB/s/core = 50us floor). Two levers cut that in half:
  * fp16 I/O: x, y uploaded as fp16; out stored as fp16 (host upcasts).
    Output error at scale=0 is exactly the fp16 rounding of x (~5e-4).
  * host-side pre-transpose: x and y are packed on the host into the
    exact transposed SBUF layouts the matmuls need ([s,c] / [s,k]),
    so every DMA is a contiguous 128-partition transfer and the PE
    never spends cycles transposing the 8MB x.

On-chip dataflow per core:
  E[c,k]   accumulated over 32 s-chunks: lhsT = xT chunk [s128,c128]
           (FWL fp16 weight loads), rhs = yT chunk [s128,k64].
  softmax  in natural [c,k] layout: DVE min-reduce, ScalarE fused
           exp(min-E) with accumulated sum, DVE reciprocal,
           att = p * (1/sum) * scale  (scale folded in -> output is
           exactly x + 0 when scale==0).
  attT     via 4 PE transposes.
  out^T    per s-chunk [s128,c512] = y_chunk^T @ attT  +  I @ xT_chunk
           (residual folded into the matmul accumulation group);
           drained PSUM->SBUF as fp16 on ScalarE (2/3) and folded via
           DVE tensor-add for 1/3 of chunks to balance engines;
           stored as packed out^T, host unpacks/transposes back.
"""

import numpy as np

N, C, K, H, W = 8, 512, 64, 64, 64
S = H * W  # 4096
SC = S // 128  # 32 s-chunks of 128
CC = C // 128  # 4 c-chunks of 128

_CACHE = {}


def pack_inputs(x_i, y_i):
    """x_i [C,S] f32, y_i [K,S] f32 -> (xt [128, SC*C], yt [128, SC*K],
    yn2 [128, SC//2*128]) all fp16.  xt[p, j*C + c] = x[c, j*128+p];
    yn2 stacks the two s-halves of y so s-chunks q and q+16 sit at
    partition rows 0-63 / 64-127 for row-tiled matmul pairs."""
    x16 = x_i.astype(np.float16).reshape(C, SC, 128)
    xt = np.ascontiguousarray(x16.transpose(2, 1, 0)).reshape(128, SC * C)
    y16 = y_i.astype(np.float16)
    yt = np.ascontiguousarray(y16.reshape(K, SC, 128).transpose(2, 1, 0)).reshape(
        128, SC * K
    )
    yn2 = np.ascontiguousarray(np.vstack([y16[:, : S // 2], y16[:, S // 2 :]]))
    return xt, yt, yn2


def unpack_output(outp):
    """outp [128, 16, 2, C] fp16 (pair q, half h -> s-chunk h*16+q)
    -> out [C, S] f32."""
    o4 = outp.reshape(128, SC // 2, 2, C)
    o3 = o4.transpose(3, 2, 1, 0)  # [c, half, q, p]
    return np.ascontiguousarray(o3).reshape(C, S).astype(np.float32)


def _build_program():
    import concourse.tile as tile
    from concourse import bacc, mybir
    from concourse.masks import make_identity

    F32 = mybir.dt.float32
    F16 = mybir.dt.float16
    AX = mybir.AxisListType
    OP = mybir.AluOpType
    AF = mybir.ActivationFunctionType

    nc = bacc.Bacc("TRN2", target_bir_lowering=False, debug=False)
    QP = SC // 2  # 16 row-tiled s-chunk pairs (q, q+16)
    xt_d = nc.dram_tensor("xt", [128, SC * C], F16, kind="ExternalInput")
    yt_d = nc.dram_tensor("yt", [128, SC * K], F16, kind="ExternalInput")
    yn_d = nc.dram_tensor("yn", [128, QP * 128], F16, kind="ExternalInput")
    s_d = nc.dram_tensor("scale", [1], F32, kind="ExternalInput")
    o_d = nc.dram_tensor("out", [128, QP, 2, C], F16, kind="ExternalOutput")

    XPIECE = 8  # xt arrives in 8 DMA pieces of 4 s-chunks (512KB) each
    JP = SC // XPIECE

    with tile.TileContext(nc) as tc:
        with (
            tc.tile_pool(name="const", bufs=1) as const,
            tc.tile_pool(name="xtp", bufs=1) as xtp,
            tc.tile_pool(name="ytp", bufs=1) as ytp,
            tc.tile_pool(name="ynp", bufs=1) as ynp,
            tc.tile_pool(name="smp", bufs=24) as smp,
            tc.tile_pool(name="attp", bufs=2) as attp,
            tc.tile_pool(name="resp", bufs=6) as resp,
            tc.tile_pool(name="e_ps", bufs=2, space="PSUM") as e_ps,
            tc.tile_pool(name="o_ps", bufs=3, space="PSUM") as o_ps,
        ):
            ident_h = const.tile([128, 128], F16)
            make_identity(nc, ident_h)
            ident_f = const.tile([128, 128], F32)
            make_identity(nc, ident_f)

            scale_sb = const.tile([128, 1], F32)
            nc.gpsimd.dma_start(out=scale_sb, in_=s_d[:].to_broadcast([128, 1]))

            # DMA order: loads split across the two HWDGE rings (SP=sync,
            # ACT=scalar) so the per-dma ~600ns issue cost and per-piece
            # completion latencies overlap.  yT first on ACT (feeds every
            # energy matmul), y-natural first on SP (its 64-partition
            # transfer drains slowly; absorbed during the ramp), then the
            # 8 xT pieces alternate rings so they complete in order.
            xt_sb = xtp.tile([128, SC * C], F16)
            yt_sb = ytp.tile([128, SC * K], F16)
            yn_sb = ynp.tile([128, QP * 128], F16)
            PW = JP * C  # columns per xt piece
            nc.scalar.dma_start(out=yt_sb[:], in_=yt_d[:])
            nc.sync.dma_start(out=yn_sb[:], in_=yn_d[:])
            for piece in range(XPIECE):
                eng = nc.scalar if piece % 2 == 0 else nc.sync
                eng.dma_start(
                    out=xt_sb[:, piece * PW : (piece + 1) * PW],
                    in_=xt_d[:, piece * PW : (piece + 1) * PW],
                )

            # prewarm ScalarE LUTs (Exp and Copy) during the DMA-idle head
            warm_in = const.tile([128, 1], F32)
            nc.vector.memset(warm_in, 0.0)
            warm = const.tile([128, 1], F32)
            nc.scalar.activation(out=warm, in_=warm_in, func=AF.Exp)
            warm2 = const.tile([128, 1], F32)
            nc.scalar.activation(out=warm2, in_=warm_in, func=AF.Copy)

            # dummy-matmul burst in the DMA-idle head: trips the PE HAM
            # activity monitor to K=8/8 (2.4GHz) before the energy stream
            wa = const.tile([128, 128], F16)
            nc.vector.memset(wa, 0.0)
            wb = const.tile([128, 512], F16)
            nc.vector.memset(wb, 0.0)
            wp = o_ps.tile([128, 2, 512], F32, tag="o_t")
            for _ in range(10):
                nc.tensor.matmul(
                    wp[:, 0, :], lhsT=wa[:], rhs=wb[:], start=True, stop=True
                )

            def filler(n, ncols=512):
                # HAM-keepalive: dense matmuls with no data deps, emitted
                # where the PE would otherwise idle (DMA waits, the softmax
                # latency chain) so it never drops to K=4/8.  N=64 fillers
                # cost ~30-55ns each and can never put the PE behind the DMA.
                f_t = o_ps.tile([128, 2, 512], F32, tag="o_t")
                for _ in range(n):
                    nc.tensor.matmul(
                        f_t[:, 0, 0:ncols], lhsT=wa[:], rhs=wb[:, 0:ncols],
                        start=True, stop=True,
                    )

            # energy: E[c128, (cc,k)] += xtT[s,c] . yt[s,k] over 32 s-chunks,
            # all 4 c-chunks accumulated side-by-side in ONE PSUM bank
            # (start=True only on the very first matmul: it clears the bank;
            # later first-writes land on cleared has_written bits and
            # overwrite, which is exactly right).
            e_all = e_ps.tile([128, CC, K], F32, tag="e")
            for j in range(SC):
                for cc in range(CC):
                    nc.tensor.matmul(
                        e_all[:, cc, :],
                        lhsT=xt_sb[:, j * C + cc * 128 : j * C + (cc + 1) * 128],
                        rhs=yt_sb[:, j * K : (j + 1) * K],
                        start=(j == 0 and cc == 0),
                        stop=(j == SC - 1),
                        skip_group_check=True,
                    )
                if j % JP == JP - 1 and j != SC - 1:
                    filler(6, 64)

            # softmax_k over each 64-wide k-row group.  Subtracting the
            # GLOBAL per-row min (over all 4 c-chunks' k values) instead of
            # the per-chunk min is algebraically identical (the normalizer
            # absorbs any per-row offset) and lets the whole chain run as a
            # handful of big ops: one reduce, ONE exp over [128,256], one
            # segmented sum, one normalize.  scale folds into att so
            # scale==0 -> att == 0 and the output is exactly fp16(x).
            filler(16, 256)  # keep PE hot through the softmax latency chain
            attT = attp.tile([128, C], F16)
            rmin4 = smp.tile([128, CC], F32, tag="sm4")
            nc.vector.tensor_reduce(out=rmin4, in_=e_all[:], axis=AX.X, op=OP.min)
            z_all = smp.tile([128, CC, K], F32, tag="z")
            nc.vector.tensor_tensor(
                out=z_all[:],
                in0=e_all[:],
                in1=rmin4[:].to_broadcast([128, CC, K]),
                op=OP.subtract,
            )
            p_all = smp.tile([128, CC, K], F32, tag="p")
            nc.scalar.activation(
                out=p_all[:],
                in_=z_all[:],
                func=AF.Exp,
                scale=-1.0,
            )
            ssum = smp.tile([128, CC], F32, tag="sm4")
            nc.vector.tensor_reduce(out=ssum, in_=p_all[:], axis=AX.X, op=OP.add)
            rcp = smp.tile([128, CC], F32, tag="sm4")
            nc.vector.reciprocal(out=rcp, in_=ssum)
            rcs = smp.tile([128, CC], F32, tag="sm4")
            nc.vector.tensor_scalar(
                out=rcs, in0=rcp, scalar1=scale_sb, scalar2=None, op0=OP.mult
            )
            att_all = smp.tile([128, CC, K], F32, tag="att")
            nc.vector.tensor_tensor(
                out=att_all[:],
                in0=p_all[:],
                in1=rcs[:].to_broadcast([128, CC, K]),
                op=OP.mult,
            )
            a_pss = []
            for cc in range(CC):
                # padded to a full PSUM bank so transposes never share a bank
                # with a tile another engine is still reading
                a_ps = e_ps.tile(
                    [K, 128], F32, name=f"a{cc}", tag="e", padded_shape=[K, 512]
                )
                nc.tensor.transpose(a_ps[:], att_all[:, cc, :], ident_f)
                a_pss.append(a_ps)
                filler(4, 256)  # adaptive PE fill while the next att chunk lands
            # attT is duplicated at partition rows 0-63 and 64-127 so the two
            # halves of each row-tiled matmul pair have their own copy;
            # assembly split DVE/ScalarE so neither serializes the tail
            for cc in range(CC):
                for h in range(2):
                    dst = attT[h * K : (h + 1) * K, cc * 128 : (cc + 1) * 128]
                    if (2 * cc + h) % 2 == 0:
                        nc.vector.tensor_copy(dst, a_pss[cc][:])
                    else:
                        nc.scalar.activation(out=dst, in_=a_pss[cc][:], func=AF.Copy)

            # out^T, one row-tiled matmul PAIR per step: s-chunks q and q+16
            # run concurrently on PE rows 0-63 / 64-127 (the K=64 contraction
            # only needs half the array), filling a 2-bank [128, 2, 512] PSUM
            # tile that is drained in ONE wide op.  Only DVE and ScalarE can
            # read PSUM, so pairs alternate: DVE folds the residual in a
            # tensor-add drain; ScalarE pairs fold it via identity matmuls
            # and drain with a plain copy.
            xt_pairs = xt_sb[:].rearrange("p (h q c) -> p h q c", h=2, q=QP, c=C)
            for q in range(QP):
                o_big = o_ps.tile([128, 2, C], F32, name=f"ob{q}", tag="o_t")
                act_drain = q % 2 == 1
                res_q = resp.tile([128, 2, C], F16, name=f"r{q}", tag="res")
                for h in range(2):
                    nc.tensor.matmul(
                        o_big[:, h, :],
                        lhsT=yn_sb[h * K : (h + 1) * K, q * 128 : (q + 1) * 128],
                        rhs=attT[h * K : (h + 1) * K, :],
                        start=True,
                        stop=not act_drain,
                        skip_group_check=True,
                    )
                if act_drain:
                    for h in range(2):
                        nc.tensor.matmul(
                            o_big[:, h, :],
                            lhsT=ident_h[:],
                            rhs=xt_sb[:, (h * QP + q) * C : (h * QP + q + 1) * C],
                            start=False,
                            stop=True,
                            skip_group_check=True,
                        )
                    nc.scalar.activation(out=res_q[:], in_=o_big[:], func=AF.Copy)
                else:
                    nc.vector.tensor_add(res_q[:], o_big[:], xt_pairs[:, :, q, :])
                nc.sync.dma_start(out=o_d[:, q, :, :], in_=res_q[:])
    nc.compile()
    return nc


def _get_program():
    if "nc" not in _CACHE:
        _CACHE["nc"] = _build_program()
    return _CACHE["nc"]


def kernel(x, y, scale):
    from concourse import bass2jax

    nc = _get_program()
    x = np.ascontiguousarray(np.asarray(x, dtype=np.float32)).reshape(N, C, S)
    y = np.ascontiguousarray(np.asarray(y, dtype=np.float32)).reshape(N, K, S)
    scale = np.ascontiguousarray(np.asarray(scale, dtype=np.float32)).reshape(1)

    in_maps = []
    for i in range(N):
        xt, yt, yn = pack_inputs(x[i], y[i])
        in_maps.append({"xt": xt, "yt": yt, "yn": yn, "scale": scale})
    results = bass2jax.run_bass_via_pjrt(nc, in_maps, n_cores=N)
    out = np.stack([unpack_output(np.asarray(results[i]["out"])) for i in range(N)])
    return out.reshape(N, C, H, W).astype(np.float32)
